# revision 23
# baseline (speedup 1.0000x reference)
"""Trainium2 Bass kernel for MultiLatentAttention (MQA with latent down-proj).

Reference computation (per batch b):
    latent = x @ Wc_down            [T, L]        T=2048, C=1024, L=512
    q      = latent @ Wq_up         [T, C] -> 16 heads x 64
    kv     = latent @ Wkv_up        [T, 128]  (k = kv[:, :64], v = kv[:, 64:], shared across heads)
    att    = softmax(causal(q k^T / 8))
    out    = (att @ v reshaped) @ Wc_proj

Sharding: 8 cores = (b in 0..3) x (head-group g in 0..1). Each core computes
latent/kv for its b (duplicated across the 2 head-group cores), q + attention
for its 8 heads, and a partial c_proj using its 512 rows of Wc_proj. The host
sums the two partials per b (the "all-reduce after c_proj", done at unshard).

Per-core layout strategy (everything transposed so PE contracts on partitions):
    xT      [C, T]   via PE transposes of x tiles
    latentT [L, T]   = Wc_down.T @ xT       (lhsT=Wc_down nat, rhs=xT)
    qT      [512, T] = Wq_up_g.T @ latentT  (2 heads per 128-partition tile)
    kvT     [128, T] = Wkv_up.T @ latentT   (k dims on partitions 0:64)
    scoresT [s, q]   = kT.T @ qT   per (head, s-block, q-chunk), K=64 row-paired
                       so both heads of a pair run concurrently in the PE array
    p = exp(scoresT/8)  on ACT; causal: upper-triangle of diagonal blocks zeroed
                       by gpsimd affine_select; fully-masked columns skipped by
                       narrowing the PV rhs.
    y_augT  [65, q]  = [v | ones].T @ p  accumulated over s-blocks in PSUM;
                       row 64 = softmax denominator l.
    normalize: r = 1/l (DVE approx recip), broadcast over partitions with a
                       K=1 ones outer-product matmul, multiply on DVE.
    c_proj: out[t, :] = sum_heads yT.T @ Wc_proj rows, K=64 row-paired.

All matmuls use float32r (full-rate PE, fp32 storage).
"""

import contextlib
import os
import numpy as np

import concourse.bass as bass
import concourse.bacc as bacc
import concourse.mybir as mybir
import concourse.tile as tile
from concourse.bass_utils import run_bass_kernel_spmd
from concourse.masks import make_identity

F32 = mybir.dt.float32
F32R = mybir.dt.float32r

B, T, C = 4, 2048, 1024
H, D = 16, 64
L = C // 2          # 512
P = 128
G = 2               # head groups (cores per batch)
HG = H // G         # heads per core = 8
CG = HG * D         # channel slice per core = 512
N_CORES = 8
SCALE = 1.0 / np.sqrt(D)  # 0.125

TB = T // P          # 16 token blocks of 128
TCH = T // 512       # 4 token chunks of 512


def r(ap):
    """Identity: tiles holding matmul inputs are allocated float32r."""
    return ap


def build_program(phases: int = 5) -> bass.Bass:
    nc = bacc.Bacc("TRN2", target_bir_lowering=False, debug=False)

    xb = nc.declare_dram_parameter("xb", [T, C], F32, isOutput=False)
    w_down = nc.declare_dram_parameter("w_down", [C, L], F32R, isOutput=False)
    w_q = nc.declare_dram_parameter("w_q", [L, CG], F32R, isOutput=False)
    w_kv = nc.declare_dram_parameter("w_kv", [L, 2 * D], F32R, isOutput=False)
    w_proj = nc.declare_dram_parameter("w_proj", [CG, C], F32R, isOutput=False)
    out = nc.declare_dram_parameter("out_part", [T, C], F32, isOutput=True)

    with tile.TileContext(nc) as tc:
        with (
            tc.tile_pool(name="const", bufs=1) as const_pool,
            tc.tile_pool(name="persist", bufs=1) as persist,
            tc.tile_pool(name="wproj", bufs=1) as wproj_pool,
        ):
            identity = const_pool.tile([P, P], F32)
            make_identity(nc, identity[:])
            ones_t = const_pool.tile([P, D], F32)
            nc.vector.memset(ones_t[:], 1.0)
            identity_r = const_pool.tile([P, P], F32R)
            nc.vector.tensor_copy(out=identity_r[:], in_=identity[:])
            # [s, q] keep-mask for diagonal blocks: 1 where q >= s else 0
            trimask_f = const_pool.tile([P, P], F32)
            nc.gpsimd.memset(trimask_f[:], 1.0)
            nc.gpsimd.affine_select(
                out=trimask_f[:],
                in_=trimask_f[:],
                pattern=[[1, P]],
                compare_op=mybir.AluOpType.is_ge,
                fill=0.0,
                base=0,
                channel_multiplier=-1,
            )
            trimask = const_pool.tile([P, P], F32R)
            nc.vector.tensor_copy(out=trimask[:], in_=trimask_f[:])

            # long-lived activations
            qT = persist.tile([P, HG // 2, T], F32R)     # [128, 4, 2048]
            kvT = persist.tile([P, T], F32R)             # k rows 0:64, v rows 64:128
            kT2 = persist.tile([P, T], F32R)             # rows 64:128 = copy of k
            vaug = persist.tile([P, TB, D + 1], F32R)    # per s-block [v | ones]
            yT = persist.tile([P, HG // 2, T], F32R)     # normalized y^T, 2 heads/tile
            if phases == 4:
                dbg_p = persist.tile([P, 1024], F32, name="dbg_p")
                dbg_y = persist.tile([P, 1024], F32, name="dbg_y")
            wp_sb = wproj_pool.tile([P, HG // 2, C], F32R)

            nc.sync.dma_start(
                wp_sb[:], w_proj.rearrange("(hp p) o -> p hp o", p=P)
            )

            # ---------------- Phase 1+2: xT and latentT ----------------
            with (
                tc.tile_pool(name="latT", bufs=1) as lat_pool,
                tc.tile_pool(name="psA", bufs=4, space="PSUM") as psA,
            ):
                if phases < 5:  # debug: keep latT alive for end-of-kernel dumps
                    latT = persist.tile([P, L // P, T], F32R, name="latT")
                else:
                    latT = lat_pool.tile([P, L // P, T], F32R)  # [128, 4, 2048]
                with (
                    tc.tile_pool(name="xload", bufs=3) as x_pool,
                    tc.tile_pool(name="xTc", bufs=2) as xT_pool,
                    tc.tile_pool(name="wdown", bufs=1) as wd_pool,
                ):
                    wd_sb = wd_pool.tile([P, C // P, L], F32R)  # [128, 8, 512]
                    nc.sync.dma_start(
                        wd_sb[:], w_down.rearrange("(c p) l -> p c l", p=P)
                    )

                    for tcn in range(TCH):
                        xTc = xT_pool.tile([P, C // P, 512], F32R)
                        for tb in range(4):
                            xt = x_pool.tile([P, C], F32)
                            row0 = tcn * 512 + tb * P
                            nc.sync.dma_start(xt[:], xb[row0 : row0 + P, :])
                            for half in range(2):
                                ps = psA.tile([P, 512], F32, tag="ps")
                                for j in range(4):
                                    ci = half * 4 + j
                                    nc.tensor.transpose(
                                        ps[:, j * P : (j + 1) * P],
                                        xt[:, ci * P : (ci + 1) * P],
                                        identity[:],
                                    )
                                nc.any.tensor_copy(
                                    out=xTc[:, half * 4 : (half + 1) * 4, tb * P : (tb + 1) * P],
                                    in_=ps[:].rearrange("p (j t) -> p j t", j=4),
                                )
                        for lc in range(L // P):
                            ps = psA.tile([P, 512], F32, tag="ps")
                            for ci in range(C // P):
                                nc.tensor.matmul(
                                    ps[:],
                                    lhsT=r(wd_sb[:, ci, lc * P : (lc + 1) * P]),
                                    rhs=r(xTc[:, ci, :]),
                                    start=(ci == 0),
                                    stop=(ci == C // P - 1),
                                )
                            nc.any.tensor_copy(
                                out=latT[:, lc, tcn * 512 : (tcn + 1) * 512], in_=ps[:]
                            )

                # ---------------- Phase 3: qT, kvT, kT2, vaug ----------------
                wq_ctx = (
                    tc.tile_pool(name="wq", bufs=1)
                    if phases >= 3
                    else contextlib.nullcontext()
                )
                with wq_ctx as wq_pool:
                  if phases >= 3:
                    wq_sb = wq_pool.tile([P, L // P, CG], F32R)  # [128, 4, 512]
                    nc.sync.dma_start(
                        wq_sb[:], w_q.rearrange("(l p) d -> p l d", p=P)
                    )
                    wkv_sb = wq_pool.tile([P, L // P, 2 * D], F32R)
                    nc.sync.dma_start(
                        wkv_sb[:], w_kv.rearrange("(l p) d -> p l d", p=P)
                    )

                    for dc in range(HG // 2):
                        for tcn in range(TCH):
                            ps = psA.tile([P, 512], F32, tag="ps")
                            for lc in range(L // P):
                                nc.tensor.matmul(
                                    ps[:],
                                    lhsT=r(wq_sb[:, lc, dc * P : (dc + 1) * P]),
                                    rhs=r(latT[:, lc, tcn * 512 : (tcn + 1) * 512]),
                                    start=(lc == 0),
                                    stop=(lc == L // P - 1),
                                )
                            nc.any.tensor_copy(
                                out=qT[:, dc, tcn * 512 : (tcn + 1) * 512], in_=ps[:]
                            )
                    for tcn in range(TCH):
                        ps = psA.tile([P, 512], F32, tag="ps")
                        for lc in range(L // P):
                            nc.tensor.matmul(
                                ps[:],
                                lhsT=r(wkv_sb[:, lc, :]),
                                rhs=r(latT[:, lc, tcn * 512 : (tcn + 1) * 512]),
                                start=(lc == 0),
                                stop=(lc == L // P - 1),
                            )
                        nc.any.tensor_copy(
                            out=kvT[:, tcn * 512 : (tcn + 1) * 512], in_=ps[:]
                        )

                    # duplicate k rows into partitions 64:128 (cross-partition -> DMA)
                    nc.sync.dma_start(kT2[D : 2 * D, :], kvT[0:D, :])
                    vtmp = wq_pool.tile([D, T], F32R)
                    nc.sync.dma_start(vtmp[:], kvT[D : 2 * D, :])

                    # v natural layout, with a trailing ones column (for softmax sums)
                    for quad in range(4):  # 4 s-blocks per psum batch
                        ps = psA.tile([P, 512], F32R, tag="psr", name="psr")
                        for j in range(4):
                            sb = quad * 4 + j
                            nc.tensor.transpose(
                                ps[:, j * D : (j + 1) * D],
                                vtmp[:, sb * P : (sb + 1) * P],
                                identity_r[0:D, 0:D],
                            )
                        nc.any.tensor_copy(
                            out=vaug[:, quad * 4 : (quad + 1) * 4, 0:D],
                            in_=ps[:, : 4 * D].rearrange("p (j d) -> p j d", j=4),
                        )
                    nc.vector.tensor_copy(
                        out=vaug[:, :, D : D + 1], in_=ones_t[:, 0:TB, None]
                    )

            # ---------------- Phase 4: attention ----------------
            with contextlib.ExitStack() as stk4:
                if phases >= 4:
                    p_pool = stk4.enter_context(tc.tile_pool(name="ptile", bufs=4))
                    r_pool = stk4.enter_context(tc.tile_pool(name="rrow", bufs=2))
                    bcs_pool = stk4.enter_context(tc.tile_pool(name="bcs", bufs=2))
                    ytmp_pool = stk4.enter_context(tc.tile_pool(name="ytmp", bufs=2))
                    psS = stk4.enter_context(tc.tile_pool(name="psS", bufs=3, space="PSUM"))
                    psY = stk4.enter_context(tc.tile_pool(name="psY", bufs=2, space="PSUM"))
                for hp in range(HG // 2) if phases >= 4 else range(0):
                    ytmp = ytmp_pool.tile([D, T], F32R)
                    for tcn in range(TCH):
                        nsb = 4 * tcn + 4  # causal s-blocks for this q-chunk
                        ys = [psY.tile([P, 512], F32, tag="y", name=f"y{s}") for s in range(2)]
                        for u in range((nsb + 1) // 2):
                            ss = [psS.tile([P, 1024], F32, tag="s", name=f"s{s}") for s in range(2)]
                            sbs = [s for s in (2 * u, 2 * u + 1) if s < nsb]
                            for w, sb in enumerate(sbs):
                                for side in range(2):  # head 2hp / 2hp+1
                                    klhs = (
                                        kvT[0:D, sb * P : (sb + 1) * P]
                                        if side == 0
                                        else kT2[D:P, sb * P : (sb + 1) * P]
                                    )
                                    nc.tensor.matmul(
                                        ss[side][:, w * 512 : (w + 1) * 512],
                                        lhsT=r(klhs),
                                        rhs=r(
                                            qT[
                                                side * D : (side + 1) * D,
                                                hp,
                                                tcn * 512 : (tcn + 1) * 512,
                                            ]
                                        ),
                                        start=True,
                                        stop=True,
                                    )
                            pts = [
                                p_pool.tile([P, 1024], F32R, tag="p", name=f"p{s}")
                                for s in range(2)
                            ]
                            width = 512 * len(sbs)
                            for side in range(2):
                                nc.scalar.activation(
                                    pts[side][:, :width],
                                    ss[side][:, :width],
                                    mybir.ActivationFunctionType.Exp,
                                    scale=float(SCALE),
                                )
                            for w, sb in enumerate(sbs):
                                j = sb - 4 * tcn  # >=0 means in-chunk
                                for side in range(2):
                                    if j >= 0:
                                        # zero strictly-upper triangle of the
                                        # diagonal block: keep where q - s >= 0
                                        blk = pts[side][
                                            :, w * 512 + j * P : w * 512 + (j + 1) * P
                                        ]
                                        nc.vector.tensor_mul(blk, blk, trimask[:])
                                    if phases == 4 and hp == 0 and tcn == 0 and u == 0 and side == 0:
                                        nc.vector.tensor_copy(
                                            out=dbg_p[:, w * 512 : (w + 1) * 512],
                                            in_=pts[side][:, w * 512 : (w + 1) * 512],
                                        )
                                    q0 = max(j, 0) * P
                                    nc.tensor.matmul(
                                        ys[side][0 : D + 1, q0:512],
                                        lhsT=r(vaug[:, sb, :]),
                                        rhs=r(pts[side][:, w * 512 + q0 : (w + 1) * 512]),
                                        start=(sb == 0),
                                        stop=(sb == nsb - 1),
                                    )
                        # normalize: r = 1/l, broadcast across partitions via
                        # K=1 ones outer product, multiply, store
                        for side in range(2):
                            rrow = r_pool.tile([P, 1024], F32, tag="r")
                            if phases == 4 and hp == 0 and tcn == 0 and side == 0:
                                nc.vector.tensor_copy(
                                    out=dbg_y[0 : D + 1, 0:512], in_=ys[side][0 : D + 1, :512]
                                )
                            nc.vector.reciprocal(
                                out=rrow[D : D + 1, 0:512], in_=ys[side][D : D + 1, :512]
                            )
                            bc = psS.tile([P, 512], F32, tag="s")
                            nc.tensor.matmul(
                                bc[0:D, :],
                                lhsT=r(ones_t[D : D + 1, :]),
                                rhs=r(rrow[D : D + 1, 0:512]),
                                start=True,
                                stop=True,
                            )
                            bcs = bcs_pool.tile([D, 512], F32, tag="b")
                            nc.any.tensor_copy(out=bcs[:], in_=bc[0:D, :])
                            if phases == 4 and hp == 0 and tcn == 0 and side == 0:
                                nc.vector.tensor_copy(
                                    out=dbg_y[D : D + 1, 512:1024], in_=rrow[D : D + 1, 0:512]
                                )
                                nc.vector.tensor_copy(
                                    out=dbg_y[0:D, 512:1024], in_=bcs[:]
                                )
                            dst = (
                                yT[0:D, hp, tcn * 512 : (tcn + 1) * 512]
                                if side == 0
                                else ytmp[:, tcn * 512 : (tcn + 1) * 512]
                            )
                            nc.vector.tensor_mul(dst, ys[side][0:D, :512], bcs[:])
                    # move odd head's yT into partitions 64:128 (cross-partition)
                    nc.sync.dma_start(yT[D:P, hp, :], ytmp[:, :])

            # ---------------- Phase 5: c_proj ----------------
            with (
                tc.tile_pool(name="outsb", bufs=3) as out_pool,
                tc.tile_pool(name="psC", bufs=4, space="PSUM") as psC,
            ):
                if phases < 5:
                    # debug dumps of intermediates into out_part rows
                    zt = out_pool.tile([P, C], F32)
                    nc.vector.memset(zt[:], 0.0)
                    for tb in range(TB):
                        nc.sync.dma_start(out[tb * P : (tb + 1) * P, :], zt[:])
                    def dump(row, ap_f32r):
                        dt_ = out_pool.tile([P, C], F32, tag="dump", name="dump")
                        nc.vector.tensor_copy(out=dt_[:, : ap_f32r.shape[-1]], in_=ap_f32r)
                        nc.sync.dma_start(out[row * P : (row + 1) * P, :], dt_[:])
                    if phases >= 2:
                        for lc in range(4):
                            dump(lc, latT[:, lc, 0:C])
                    if phases >= 3:
                        for dc in range(4):
                            dump(4 + dc, qT[:, dc, 0:C])
                        dump(8, kvT[:, 0:C])
                        dump(9, kT2[:, 0:C])
                        dump(10, vaug[:, 0:15, :].rearrange("p a b -> p (a b)"))
                    if phases >= 4:
                        for hp in range(3):
                            dump(11 + hp, yT[:, hp, 0:C])
                        dt14 = out_pool.tile([P, C], F32, tag="dump", name="dump14")
                        nc.vector.tensor_copy(out=dt14[:], in_=dbg_y[:])
                        nc.sync.dma_start(out[14 * P : 15 * P, :], dt14[:])
                        dt15 = out_pool.tile([P, C], F32, tag="dump", name="dump15")
                        nc.vector.tensor_copy(out=dt15[:], in_=dbg_p[:])
                        nc.sync.dma_start(out[15 * P : 16 * P, :], dt15[:])
                for tb in range(TB) if phases >= 5 else range(0):
                    osb = out_pool.tile([P, C], F32)
                    for oc in range(C // 512):
                        ps = psC.tile([P, 512], F32, tag="c")
                        for hp in range(HG // 2):
                            # both heads of the pair are stacked on the same
                            # 128 partitions -> one K=128 matmul contracts both
                            nc.tensor.matmul(
                                ps[:],
                                lhsT=r(yT[:, hp, tb * P : (tb + 1) * P]),
                                rhs=r(wp_sb[:, hp, oc * 512 : (oc + 1) * 512]),
                                start=(hp == 0),
                                stop=(hp == HG // 2 - 1),
                            )
                        nc.any.tensor_copy(out=osb[:, oc * 512 : (oc + 1) * 512], in_=ps[:])
                    nc.sync.dma_start(out[tb * P : (tb + 1) * P, :], osb[:])

    nc.compile()
    return nc


_PROGRAM = None
LAST_RESULTS = None


def kernel(x, Wc_down, Wq_up, Wkv_up, Wc_proj):
    global _PROGRAM, LAST_RESULTS
    x = np.ascontiguousarray(np.asarray(x, dtype=np.float32))
    Wc_down = np.ascontiguousarray(np.asarray(Wc_down, dtype=np.float32))
    Wq_up = np.ascontiguousarray(np.asarray(Wq_up, dtype=np.float32))
    Wkv_up = np.ascontiguousarray(np.asarray(Wkv_up, dtype=np.float32))
    Wc_proj = np.ascontiguousarray(np.asarray(Wc_proj, dtype=np.float32))

    if _PROGRAM is None:
        _PROGRAM = build_program()

    in_maps = []
    for core in range(N_CORES):
        b, g = core // G, core % G
        in_maps.append(
            {
                "xb": x[b],
                "w_down": Wc_down,
                "w_q": np.ascontiguousarray(Wq_up[:, g * CG : (g + 1) * CG]),
                "w_kv": Wkv_up,
                "w_proj": np.ascontiguousarray(Wc_proj[g * CG : (g + 1) * CG, :]),
            }
        )

    LAST_RESULTS = run_bass_kernel_spmd(_PROGRAM, in_maps, list(range(N_CORES)))
    res = LAST_RESULTS.results
    out = np.empty((B, T, C), dtype=np.float32)
    for b in range(B):
        out[b] = res[G * b]["out_part"] + res[G * b + 1]["out_part"]
    return out
